# revision 20
# baseline (speedup 1.0000x reference)
"""Trainium2 Bass kernel for nn_DotProcessorBlock (v9).

Computes, for x:[B,N] f32 (B=4096, N=256), w,b:[N]:
    feat = x * w + b
    Z[b,i,j] = feat[b,i] * feat[b,j]
    out = Z.reshape(B, N*N)[:, :N*(N+1)//2]   -> [4096, 32896]

Sharding: data-parallel batch split across 8 NeuronCores (512 rows each,
4 tiles of 128 partitions); w/b replicated.

Kept pairs are exactly {a,b: min(a,b) <= 127}; the device computes each
unique product once as 128 row-suffixes: row a covers columns
[j0(a), 256), j0(a) = a - a%2. Host reconstructs the full output by a
pure gather + dtype cast.

Engine/precision split (HW-measured):
- Host permutes the 128 "lo" features ascending by w^2+b^2 so the
  longest rows are the lowest-energy ones. Rows 0..67 ship as TRN fp8e4
  (range-exact: max |prod| in fp8 rows ~39 << 240); measured norm rel
  err ~1.1e-2 vs the 2e-2 gate.
- ACT: rows 0..19 direct to fp8 (f32 in, ~581 ns/row) plus one bulk
  bf16->fp8 convert of group g0 (rows 20..27), lagged one tile so it
  never blocks the ACT queue.
- DVE: feat chain + rows 20..127 as 13 r=8 groups and one r=4 group of
  tensor_tensor bf16 (2x_1P, ~0.52 ns/elem + 166 ns/op):
  out[p,j,r] = frep8[p,8(i0+j)+r] * fb16[p,i0+r].
- Groups g1..g5 (rows 28..67) are written via SWDGE cast-DMA
  (gpsimd queue): SDMA converts bf16->fp8e4 in flight (bit-exact RNE,
  HW-verified). This halves those chunks' HBM-side bytes; the two cores
  of an HBM pair share ~716 GB/s, and cutting HBM demand below that
  contended ceiling is what removes the slow-mode runs (engine times
  are identical between fast/slow runs; only DMA time varies).
- POOL: only the SWDGE descriptor generation for cast chunks.
- x tiles and all other output ride HWDGE (sync), chunks issued in
  completion order.
"""

from contextlib import ExitStack

import numpy as np

import concourse.bacc as bacc
import concourse.tile as tile
from concourse import mybir
from concourse.bass_utils import run_bass_kernel_spmd

B_FULL = 4096
N = 256
N_LO = 128
N_CORES = 8
B_CORE = B_FULL // N_CORES          # 512
NUM_INTS = N * (N + 1) // 2         # 32896
P = 128
N_BT = B_CORE // P                  # 4 batch tiles per core

FP32 = mybir.dt.float32
BF16 = mybir.dt.bfloat16
F8E4 = mybir.dt.float8e4

N_ACT = 20                          # rows 0..19 on ACT, fp8 direct
RQ = 8                              # frep interleave factor
N_CVT = 1                           # groups converted bf16->fp8 by ACT
CAST_G = range(1, 6)                # groups shipped via SWDGE cast-DMA

_J0 = [i - (i % 2) for i in range(P)]

# DVE groups: (i0, r) covering rows N_ACT..127
GROUPS = []
_i0 = N_ACT
while _i0 < P:
    r = min(RQ, P - _i0)
    GROUPS.append((_i0, r))
    _i0 += r
N_GRP = len(GROUPS)
_GRP_LEN = [r * (N - i0) for i0, r in GROUPS]

# ---- fp8 zone: ACT rows | cvt group | cast groups ----
_ACT_OFF = np.zeros(N_ACT, np.int64)
_off = 0
for _i in range(N_ACT):
    _ACT_OFF[_i] = _off
    _off += N - _J0[_i]
_CVT_OFF = np.zeros(N_CVT, np.int64)
for _g in range(N_CVT):
    _CVT_OFF[_g] = _off
    _off += _GRP_LEN[_g]
_CAST_OFF = {}
for _g in CAST_G:
    _CAST_OFF[_g] = _off
    _off += _GRP_LEN[_g]
C_F8 = int(_off)
# ---- bf16 zone: remaining groups ----
_B16_OFF = {}
_off = 0
for _g in range(N_GRP):
    if _g < N_CVT or _g in _CAST_OFF:
        continue
    _B16_OFF[_g] = _off
    _off += _GRP_LEN[_g]
C_B16 = int(_off)
B16_GRPS = sorted(_B16_OFF)         # [6..13]


def _grp_of_row(a):
    g = (a - N_ACT) // RQ
    return min(g, N_GRP - 1)


def _pair_col(a, b):
    """Column in the combined [f8 | b16] space holding Z[a, b], a<=b."""
    if a < N_ACT:
        return int(_ACT_OFF[a]) + (b - _J0[a])
    g = _grp_of_row(a)
    i0, r = GROUPS[g]
    if g < N_CVT:
        return int(_CVT_OFF[g]) + (b - i0) * r + (a - i0)
    if g in _CAST_OFF:
        return int(_CAST_OFF[g]) + (b - i0) * r + (a - i0)
    return C_F8 + int(_B16_OFF[g]) + (b - i0) * r + (a - i0)


def _build_src(perm_lo):
    inv = np.empty(N_LO, np.int64)
    inv[perm_lo] = np.arange(N_LO)
    src = np.empty(NUM_INTS, np.int64)
    for c in range(NUM_INTS):
        i, j = divmod(c, N)
        if i >= N_LO:           # tail row 128: pair (j<128, 128)
            i, j = j, i
        pi = inv[i]
        qj = inv[j] if j < N_LO else j
        a, b = (pi, qj) if pi <= qj else (qj, pi)
        src[c] = _pair_col(a, b)
    return src


# ---- DMA chunk plans (issue order ~ completion order) ----
# sync queue: ("C",) lagged cvt; ("A", r0, r1) ACT rows; ("G", gs...) groups
# gpsimd queue: ("D", gs...) cast groups
_CHUNKS_SYNC = [
    ("C",), ("A", 0, 10), ("G", 6, 8), ("A", 10, N_ACT),
    ("G", 8, 10), ("G", 10, 12), ("G", 12, N_GRP),
]
_CHUNKS_CAST = [("D", 1, 3), ("D", 3, 6)]


def _zone_span(gs, table):
    c0 = int(table[gs[0]])
    end = int(table[gs[-1]]) + _GRP_LEN[gs[-1]]
    assert end - c0 == sum(_GRP_LEN[g] for g in gs)
    return c0, end - c0


def _chunk_cols(ch):
    k = ch[0]
    if k == "A":
        c0 = int(_ACT_OFF[ch[1]])
        end = int(_ACT_OFF[ch[2]]) if ch[2] < N_ACT else int(_CVT_OFF[0])
        return c0, end - c0
    if k == "C":
        return int(_CVT_OFF[0]), _GRP_LEN[0]
    if k == "D":
        return _zone_span(range(ch[1], ch[2]), _CAST_OFF)
    return _zone_span(range(ch[1], ch[2]), _B16_OFF)


def _emit(ctx, tc, cout_f8, cout_b16, wb, xr):
    nc = tc.nc
    const_pool = ctx.enter_context(tc.tile_pool(name="const", bufs=1))
    x_pool = ctx.enter_context(tc.tile_pool(name="x", bufs=2))
    f_pool = ctx.enter_context(tc.tile_pool(name="feat", bufs=2))
    fb_pool = ctx.enter_context(tc.tile_pool(name="featb", bufs=2))
    fr_pool = ctx.enter_context(tc.tile_pool(name="frep", bufs=2))
    a_pool = ctx.enter_context(tc.tile_pool(name="actz", bufs=2))
    cv_pool = ctx.enter_context(tc.tile_pool(name="cvt", bufs=2))
    o_pool = ctx.enter_context(tc.tile_pool(name="out", bufs=10))

    wb_t = const_pool.tile([P, 2 * N], FP32, tag="wb")
    nc.sync.dma_start(wb_t[:], wb[:])
    w_t = wb_t[:, 0:N]
    b_t = wb_t[:, N:2 * N]
    # Prepay ACT's activation-table load off the critical path.
    warm = const_pool.tile([P, 2], FP32, tag="warm")
    nc.scalar.mul(warm[:], wb_t[:, 0:2], wb_t[:, 0:1])

    def x_load(bt):
        x_t = x_pool.tile([P, N], FP32, tag="x")
        nc.sync.dma_start(x_t[:], xr[bt * P:(bt + 1) * P, :])
        return x_t

    def feat_chain(x_t):
        feat = f_pool.tile([P, N], FP32, tag="feat")
        fb16 = fb_pool.tile([P, N], BF16, tag="fb16")
        frep8 = fr_pool.tile([P, RQ * N], BF16, tag="frep8")
        nc.vector.tensor_mul(feat[:], x_t[:], w_t)
        nc.vector.tensor_add(feat[:], feat[:], b_t)
        nc.vector.tensor_copy(fb16[:], feat[:])
        nc.vector.tensor_copy(
            frep8[:].rearrange("p (k r) -> p k r", k=N, r=RQ),
            fb16[:].unsqueeze(2).broadcast_to((P, N, RQ)))
        return feat, fb16, frep8

    def grp_op(fb16, frep8, g, dst, doff):
        i0, r = GROUPS[g]
        Lg = N - i0
        out3 = dst[:, doff:doff + r * Lg].rearrange(
            "p (j r) -> p j r", j=Lg, r=r)
        in0 = frep8[:, RQ * i0:RQ * N].rearrange(
            "p (j rr) -> p j rr", j=Lg, rr=RQ)[:, :, 0:r]
        in1 = fb16[:, i0:i0 + r].unsqueeze(1).broadcast_to((P, Lg, r))
        nc.vector.tensor_mul(out3, in0, in1)

    nxt = feat_chain(x_load(0))
    nxt_x = None
    prev_cvt = None                 # (cvt_dst, cvt_src, tile index)
    for bt in range(N_BT):
        feat, fb16, frep8 = nxt
        last = bt + 1 == N_BT

        cvt_src = cv_pool.tile([P, _GRP_LEN[0]], BF16, tag="cvsrc")
        cvt_dst = cv_pool.tile([P, _GRP_LEN[0]], F8E4, tag="cvdst")
        if prev_cvt is not None:
            pdst, psrc, pbt = prev_cvt
            nc.scalar.copy(pdst[:], psrc[:])

        # -- DVE: cvt-source group, then cast groups, then the rest --
        grp_op(fb16, frep8, 0, cvt_src, 0)
        if not last:
            nxt_x = x_load(bt + 1)

        rows = slice(bt * P, (bt + 1) * P)
        cast_tiles = []
        for ch in _CHUNKS_CAST:
            c0, csz = _chunk_cols(ch)
            ot = o_pool.tile([P, csz], BF16, tag="otc")
            for g in range(ch[1], ch[2]):
                grp_op(fb16, frep8, g, ot, int(_CAST_OFF[g]) - c0)
            cast_tiles.append((ch, ot))
            # SWDGE cast bf16 -> fp8e4 in flight
            nc.gpsimd.dma_start(cout_f8[rows, c0:c0 + csz], ot[:, :csz])

        chunk_tiles = {}
        for ch in _CHUNKS_SYNC:
            if ch[0] != "G":
                continue
            c0, csz = _chunk_cols(ch)
            ot = o_pool.tile([P, csz], BF16, tag="ot")
            chunk_tiles[ch] = ot
            for g in range(ch[1], ch[2]):
                grp_op(fb16, frep8, g, ot, int(_B16_OFF[g]) - c0)

        # -- ACT: direct fp8 rows (last tile: un-lagged cvt mid-stream) --
        act_t = a_pool.tile([P, int(_CVT_OFF[0])], F8E4, tag="actz")

        def act_rows(i0, i1):
            for i in range(i0, i1):
                o0 = int(_ACT_OFF[i])
                L = N - _J0[i]
                nc.scalar.mul(act_t[:, o0:o0 + L],
                              feat[:, _J0[i]:N], feat[:, i:i + 1])

        if last:
            act_rows(0, 10)
            nc.scalar.copy(cvt_dst[:], cvt_src[:])
            act_rows(10, N_ACT)
        else:
            act_rows(0, N_ACT)

        # -- DVE: next tile's feat chain --
        if not last:
            nxt = feat_chain(nxt_x)

        # -- sync-queue DMA in completion order --
        for ch in _CHUNKS_SYNC:
            c0, csz = _chunk_cols(ch)
            if ch[0] == "A":
                nc.sync.dma_start(cout_f8[rows, c0:c0 + csz],
                                  act_t[:, c0:c0 + csz])
            elif ch[0] == "C":
                if prev_cvt is not None:
                    pdst, psrc, pbt = prev_cvt
                    prows = slice(pbt * P, (pbt + 1) * P)
                    nc.sync.dma_start(cout_f8[prows, c0:c0 + csz], pdst[:])
                if last:
                    nc.sync.dma_start(cout_f8[rows, c0:c0 + csz],
                                      cvt_dst[:])
            else:
                nc.sync.dma_start(cout_b16[rows, c0:c0 + csz],
                                  chunk_tiles[ch][:, :csz])
        prev_cvt = (cvt_dst, cvt_src, bt)


def _build():
    nc = bacc.Bacc("TRN2", target_bir_lowering=False, debug=False,
                   num_devices=N_CORES)
    wb = nc.dram_tensor("wb", [P, 2 * N], FP32, kind="ExternalInput").ap()
    xr = nc.dram_tensor("xr", [B_CORE, N], FP32, kind="ExternalInput").ap()
    cout_f8 = nc.dram_tensor("cout_f8", [B_CORE, C_F8], F8E4,
                             kind="ExternalOutput").ap()
    cout_b16 = nc.dram_tensor("cout_b16", [B_CORE, C_B16], BF16,
                              kind="ExternalOutput").ap()
    with tile.TileContext(nc) as tc, ExitStack() as ctx:
        _emit(ctx, tc, cout_f8, cout_b16, wb, xr)
    nc.compile()
    return nc


_NC_CACHE = None


def _get_nc():
    global _NC_CACHE
    if _NC_CACHE is None:
        _NC_CACHE = _build()
    return _NC_CACHE


def run(x, weight_w, weight_b, trace=False, **run_kwargs):
    x = np.ascontiguousarray(np.asarray(x, dtype=np.float32))
    w = np.asarray(weight_w, dtype=np.float32).reshape(N)
    b = np.asarray(weight_b, dtype=np.float32).reshape(N)
    assert x.shape == (B_FULL, N), x.shape

    energy = w[:N_LO] ** 2 + b[:N_LO] ** 2
    perm_lo = np.argsort(energy, kind="stable")
    perm = np.concatenate([perm_lo, np.arange(N_LO, N)])
    xp = np.ascontiguousarray(x[:, perm])
    wp, bp = w[perm], b[perm]
    src = _build_src(perm_lo)

    wb = np.ascontiguousarray(
        np.broadcast_to(np.concatenate([wp, bp]), (P, 2 * N)))
    in_maps = []
    for i in range(N_CORES):
        in_maps.append({
            "wb": wb,
            "xr": xp[i * B_CORE:(i + 1) * B_CORE],
        })
    res = run_bass_kernel_spmd(
        _get_nc(), in_maps, core_ids=list(range(N_CORES)), trace=trace,
        **run_kwargs,
    )
    f8 = np.concatenate([r["cout_f8"] for r in res.results], axis=0)
    b16 = np.concatenate([r["cout_b16"] for r in res.results], axis=0)
    assert f8.shape == (B_FULL, C_F8) and b16.shape == (B_FULL, C_B16)
    vals = np.empty((B_FULL, C_F8 + C_B16), np.float32)
    vals[:, :C_F8] = f8.astype(np.float32)
    vals[:, C_F8:] = b16.astype(np.float32)
    full = vals[:, src]
    return full, res


def kernel(x, weight_w, weight_b):
    full, _ = run(x, weight_w, weight_b, trace=False)
    return full


# revision 21
# speedup vs baseline: 1.0427x; 1.0427x over previous
"""Trainium2 Bass kernel for nn_DotProcessorBlock (v8).

Computes, for x:[B,N] f32 (B=4096, N=256), w,b:[N]:
    feat = x * w + b
    Z[b,i,j] = feat[b,i] * feat[b,j]
    out = Z.reshape(B, N*N)[:, :N*(N+1)//2]   -> [4096, 32896]

Sharding: data-parallel batch split across 8 NeuronCores (512 rows each,
4 tiles of 128 partitions); w/b replicated.

Kept pairs are exactly {a,b: min(a,b) <= 127}; the device computes each
unique product once as 128 row-suffixes: row a covers columns
[j0(a), 256), j0(a) = a - a%2. Host reconstructs the full output by a
pure gather + dtype cast.

Engine/precision split (HW-measured):
- Host permutes the 128 "lo" features ascending by w^2+b^2 so the
  longest rows are the lowest-energy ones. fp8 assignments therefore
  carry little output energy: measured rel err ~8e-3 vs the 2e-2 gate.
- ACT: rows 0..19 directly to fp8e4 (f32 in, ~581 ns/row) plus one bulk
  bf16->fp8 convert of group g0 (rows 20..27), lagged one tile so it
  never blocks the ACT queue.
- DVE: feat chain + rows 20..127 as 13 r=8 groups and one r=4 group of
  tensor_tensor bf16 (2x_1P, ~0.52 ns/elem + 166 ns/op):
  out[p,j,r] = frep8[p,8(i0+j)+r] * fb16[p,i0+r].
- All DMA is HWDGE on the sync queue (SWDGE's Q7 descriptor rings
  congest SDMA engines 7/15 and cause straggler tails); x tiles are
  prefetched one tile ahead on the same queue.
- The kernel is DMA-bound in steady state (~400 GB/s SDMA): the LAST
  tile writes groups g1..g4 directly in fp8 from DVE (1x mode, ~1.04
  ns/elem - slower, but DVE is otherwise idle while the final DMA
  drains), trimming ~0.9 MB off the trailing bytes.
- Output DMA chunks are issued in completion order.
"""

from contextlib import ExitStack

import numpy as np

import concourse.bacc as bacc
import concourse.tile as tile
from concourse import mybir
from concourse.bass_utils import run_bass_kernel_spmd

B_FULL = 4096
N = 256
N_LO = 128
N_CORES = 8
B_CORE = B_FULL // N_CORES          # 512
NUM_INTS = N * (N + 1) // 2         # 32896
P = 128
N_BT = B_CORE // P                  # 4 batch tiles per core

FP32 = mybir.dt.float32
BF16 = mybir.dt.bfloat16
F8E4 = mybir.dt.float8e4

N_ACT = 20                          # rows 0..19 on ACT, fp8 direct
RQ = 8                              # frep interleave factor
N_CVT = 1                           # groups converted bf16->fp8 (g0)
DIR_G = range(1, 5)                 # last tile: groups written fp8 by DVE

_J0 = [i - (i % 2) for i in range(P)]

# DVE groups: (i0, r) covering rows N_ACT..127
GROUPS = []
_i0 = N_ACT
while _i0 < P:
    r = min(RQ, P - _i0)
    GROUPS.append((_i0, r))
    _i0 += r
N_GRP = len(GROUPS)
_GRP_LEN = [r * (N - i0) for i0, r in GROUPS]

# ---- fp8 zone: ACT rows, cvt group, then last-tile direct groups ----
_ACT_OFF = np.zeros(N_ACT, np.int64)
_off = 0
for _i in range(N_ACT):
    _ACT_OFF[_i] = _off
    _off += N - _J0[_i]
_CVT_OFF = np.zeros(N_CVT, np.int64)
for _g in range(N_CVT):
    _CVT_OFF[_g] = _off
    _off += _GRP_LEN[_g]
C_F8_COMMON = int(_off)
_DIR_OFF = {}
for _g in DIR_G:
    _DIR_OFF[_g] = _off
    _off += _GRP_LEN[_g]
C_F8 = int(_off)
# ---- bf16 zone: all non-cvt groups (last tile skips DIR_G ones) ----
_B16_OFF = np.zeros(N_GRP, np.int64)
_off = 0
for _g in range(N_CVT, N_GRP):
    _B16_OFF[_g] = _off
    _off += _GRP_LEN[_g]
C_B16 = int(_off)


def _grp_of_row(a):
    g = (a - N_ACT) // RQ
    return min(g, N_GRP - 1)


def _pair_col(a, b, last_tile):
    """Column in the combined [f8 | b16] space holding Z[a, b], a<=b."""
    if a < N_ACT:
        return int(_ACT_OFF[a]) + (b - _J0[a])
    g = _grp_of_row(a)
    i0, r = GROUPS[g]
    if g < N_CVT:
        return int(_CVT_OFF[g]) + (b - i0) * r + (a - i0)
    if last_tile and g in _DIR_OFF:
        return int(_DIR_OFF[g]) + (b - i0) * r + (a - i0)
    return C_F8 + int(_B16_OFF[g]) + (b - i0) * r + (a - i0)


def _build_src(perm_lo):
    """src[t][c]: combined-space index per batch tile t (last differs)."""
    inv = np.empty(N_LO, np.int64)
    inv[perm_lo] = np.arange(N_LO)
    srcs = []
    for last in (False, True):
        src = np.empty(NUM_INTS, np.int64)
        for c in range(NUM_INTS):
            i, j = divmod(c, N)
            if i >= N_LO:       # tail row 128: pair (j<128, 128)
                i, j = j, i
            pi = inv[i]
            qj = inv[j] if j < N_LO else j
            a, b = (pi, qj) if pi <= qj else (qj, pi)
            src[c] = _pair_col(a, b, last)
        srcs.append(src)
    return srcs


# ---- DMA chunk plans (issue order ~ completion order) ----
_CHUNKS = [
    ("C", 0), ("G", 1, 3), ("A", 0, 10), ("G", 3, 5), ("G", 5, 7),
    ("A", 10, N_ACT), ("G", 7, 9), ("G", 9, 11), ("G", 11, N_GRP),
]
_CHUNKS_LAST = [
    ("C", 0), ("D", 1, 3), ("A", 0, 10), ("D", 3, 5), ("CV",),
    ("G", 5, 7), ("A", 10, N_ACT), ("G", 7, 9), ("G", 9, 11),
    ("G", 11, N_GRP),
]


def _chunk_cols(ch):
    k = ch[0]
    if k == "A":
        c0 = int(_ACT_OFF[ch[1]])
        end = int(_ACT_OFF[ch[2]]) if ch[2] < N_ACT else int(_CVT_OFF[0])
        return c0, end - c0
    if k == "C" or k == "CV":
        return int(_CVT_OFF[0]), _GRP_LEN[0]
    if k == "D":
        c0 = int(_DIR_OFF[ch[1]])
        end = int(_DIR_OFF[ch[2]]) if ch[2] in _DIR_OFF else C_F8
        return c0, end - c0
    c0 = int(_B16_OFF[ch[1]])
    end = int(_B16_OFF[ch[2]]) if ch[2] < N_GRP else C_B16
    return c0, end - c0


def _emit(ctx, tc, cout_f8, cout_b16, wb, xr):
    nc = tc.nc
    const_pool = ctx.enter_context(tc.tile_pool(name="const", bufs=1))
    x_pool = ctx.enter_context(tc.tile_pool(name="x", bufs=2))
    f_pool = ctx.enter_context(tc.tile_pool(name="feat", bufs=2))
    fb_pool = ctx.enter_context(tc.tile_pool(name="featb", bufs=2))
    fr_pool = ctx.enter_context(tc.tile_pool(name="frep", bufs=2))
    a_pool = ctx.enter_context(tc.tile_pool(name="actz", bufs=2))
    cv_pool = ctx.enter_context(tc.tile_pool(name="cvt", bufs=2))
    o_pool = ctx.enter_context(tc.tile_pool(name="out", bufs=10))

    wb_t = const_pool.tile([P, 2 * N], FP32, tag="wb")
    nc.sync.dma_start(wb_t[:], wb[:])
    w_t = wb_t[:, 0:N]
    b_t = wb_t[:, N:2 * N]
    # Prepay ACT's activation-table load off the critical path.
    warm = const_pool.tile([P, 2], FP32, tag="warm")
    nc.scalar.mul(warm[:], wb_t[:, 0:2], wb_t[:, 0:1])

    def x_load(bt):
        """HWDGE x tile load (no SWDGE anywhere: its Q7 descriptor rings
        congest SDMA engines 7/15 and cause straggler tails)."""
        x_t = x_pool.tile([P, N], FP32, tag="x")
        nc.sync.dma_start(x_t[:], xr[bt * P:(bt + 1) * P, :])
        return x_t

    def feat_chain(x_t):
        feat = f_pool.tile([P, N], FP32, tag="feat")
        fb16 = fb_pool.tile([P, N], BF16, tag="fb16")
        frep8 = fr_pool.tile([P, RQ * N], BF16, tag="frep8")
        nc.vector.tensor_mul(feat[:], x_t[:], w_t)
        nc.vector.tensor_add(feat[:], feat[:], b_t)
        nc.vector.tensor_copy(fb16[:], feat[:])
        nc.vector.tensor_copy(
            frep8[:].rearrange("p (k r) -> p k r", k=N, r=RQ),
            fb16[:].unsqueeze(2).broadcast_to((P, N, RQ)))
        return feat, fb16, frep8

    def grp_op(fb16, frep8, g, dst, doff):
        i0, r = GROUPS[g]
        Lg = N - i0
        out3 = dst[:, doff:doff + r * Lg].rearrange(
            "p (j r) -> p j r", j=Lg, r=r)
        in0 = frep8[:, RQ * i0:RQ * N].rearrange(
            "p (j rr) -> p j rr", j=Lg, rr=RQ)[:, :, 0:r]
        in1 = fb16[:, i0:i0 + r].unsqueeze(1).broadcast_to((P, Lg, r))
        nc.vector.tensor_mul(out3, in0, in1)

    nxt = feat_chain(x_load(0))
    nxt_x = None                    # x tile for bt+1, loaded mid-tile
    prev_cvt = None                 # (cvt_dst, cvt_src, tile index)
    for bt in range(N_BT):
        feat, fb16, frep8 = nxt
        last = bt + 1 == N_BT
        plan = _CHUNKS_LAST if last else _CHUNKS

        cvt_src = cv_pool.tile([P, _GRP_LEN[0]], BF16, tag="cvsrc")
        cvt_dst = cv_pool.tile([P, _GRP_LEN[0]], F8E4, tag="cvdst")
        if prev_cvt is not None:
            pdst, psrc, pbt = prev_cvt
            nc.scalar.copy(pdst[:], psrc[:])

        # -- DVE: cvt-source group first, then the rest --
        grp_op(fb16, frep8, 0, cvt_src, 0)
        if not last:
            nxt_x = x_load(bt + 1)

        chunk_tiles = {}
        for ch in plan:
            if ch[0] == "G":
                c0, csz = _chunk_cols(ch)
                ot = o_pool.tile([P, csz], BF16, tag="ot")
                chunk_tiles[ch] = ot
                for g in range(ch[1], ch[2]):
                    if last and g in _DIR_OFF:
                        continue
                    grp_op(fb16, frep8, g, ot, int(_B16_OFF[g]) - c0)
            elif ch[0] == "D":
                c0, csz = _chunk_cols(ch)
                ot = o_pool.tile([P, csz], F8E4, tag="otf8")
                chunk_tiles[ch] = ot
                for g in range(ch[1], ch[2]):
                    grp_op(fb16, frep8, g, ot, int(_DIR_OFF[g]) - c0)

        # -- ACT: direct fp8 rows (last tile: un-lagged cvt mid-stream) --
        act_t = a_pool.tile([P, int(_CVT_OFF[0])], F8E4, tag="actz")

        def act_rows(i0, i1):
            for i in range(i0, i1):
                o0 = int(_ACT_OFF[i])
                L = N - _J0[i]
                nc.scalar.mul(act_t[:, o0:o0 + L],
                              feat[:, _J0[i]:N], feat[:, i:i + 1])

        if last:
            act_rows(0, 10)
            nc.scalar.copy(cvt_dst[:], cvt_src[:])
            act_rows(10, N_ACT)
        else:
            act_rows(0, N_ACT)

        # -- DVE: next tile's feat chain --
        if not last:
            nxt = feat_chain(nxt_x)

        # -- DMA in completion order --
        rows = slice(bt * P, (bt + 1) * P)
        for ch in plan:
            c0, csz = _chunk_cols(ch)
            if ch[0] == "A":
                nc.sync.dma_start(cout_f8[rows, c0:c0 + csz],
                                  act_t[:, c0:c0 + csz])
            elif ch[0] == "C":
                if prev_cvt is not None:
                    pdst, psrc, pbt = prev_cvt
                    prows = slice(pbt * P, (pbt + 1) * P)
                    nc.sync.dma_start(cout_f8[prows, c0:c0 + csz], pdst[:])
            elif ch[0] == "CV":
                nc.sync.dma_start(cout_f8[rows, c0:c0 + csz], cvt_dst[:])
            elif ch[0] == "D":
                nc.sync.dma_start(cout_f8[rows, c0:c0 + csz],
                                  chunk_tiles[ch][:, :csz])
            else:
                nc.sync.dma_start(cout_b16[rows, c0:c0 + csz],
                                  chunk_tiles[ch][:, :csz])
        prev_cvt = (cvt_dst, cvt_src, bt)


def _build():
    nc = bacc.Bacc("TRN2", target_bir_lowering=False, debug=False,
                   num_devices=N_CORES)
    wb = nc.dram_tensor("wb", [P, 2 * N], FP32, kind="ExternalInput").ap()
    xr = nc.dram_tensor("xr", [B_CORE, N], FP32, kind="ExternalInput").ap()
    cout_f8 = nc.dram_tensor("cout_f8", [B_CORE, C_F8], F8E4,
                             kind="ExternalOutput").ap()
    cout_b16 = nc.dram_tensor("cout_b16", [B_CORE, C_B16], BF16,
                              kind="ExternalOutput").ap()
    with tile.TileContext(nc) as tc, ExitStack() as ctx:
        _emit(ctx, tc, cout_f8, cout_b16, wb, xr)
    nc.compile()
    return nc


_NC_CACHE = None


def _get_nc():
    global _NC_CACHE
    if _NC_CACHE is None:
        _NC_CACHE = _build()
    return _NC_CACHE


def run(x, weight_w, weight_b, trace=False, **run_kwargs):
    x = np.ascontiguousarray(np.asarray(x, dtype=np.float32))
    w = np.asarray(weight_w, dtype=np.float32).reshape(N)
    b = np.asarray(weight_b, dtype=np.float32).reshape(N)
    assert x.shape == (B_FULL, N), x.shape

    energy = w[:N_LO] ** 2 + b[:N_LO] ** 2
    perm_lo = np.argsort(energy, kind="stable")
    perm = np.concatenate([perm_lo, np.arange(N_LO, N)])
    xp = np.ascontiguousarray(x[:, perm])
    wp, bp = w[perm], b[perm]
    src_mid, src_last = _build_src(perm_lo)

    wb = np.ascontiguousarray(
        np.broadcast_to(np.concatenate([wp, bp]), (P, 2 * N)))
    in_maps = []
    for i in range(N_CORES):
        in_maps.append({
            "wb": wb,
            "xr": xp[i * B_CORE:(i + 1) * B_CORE],
        })
    res = run_bass_kernel_spmd(
        _get_nc(), in_maps, core_ids=list(range(N_CORES)), trace=trace,
        **run_kwargs,
    )
    f8 = np.concatenate([r["cout_f8"] for r in res.results], axis=0)
    b16 = np.concatenate([r["cout_b16"] for r in res.results], axis=0)
    assert f8.shape == (B_FULL, C_F8) and b16.shape == (B_FULL, C_B16)
    vals = np.empty((B_FULL, C_F8 + C_B16), np.float32)
    vals[:, :C_F8] = f8.astype(np.float32)
    vals[:, C_F8:] = b16.astype(np.float32)
    # per-tile gather: the last 128-row tile of each core's 512-row shard
    # uses the direct-fp8 layout
    v4 = vals.reshape(N_CORES, N_BT, P, C_F8 + C_B16)
    full = np.empty((N_CORES, N_BT, P, NUM_INTS), np.float32)
    full[:, :N_BT - 1] = v4[:, :N_BT - 1][..., src_mid]
    full[:, N_BT - 1] = v4[:, N_BT - 1][..., src_last]
    return full.reshape(B_FULL, NUM_INTS), res


def kernel(x, weight_w, weight_b):
    full, _ = run(x, weight_w, weight_b, trace=False)
    return full
